# revision 8
# baseline (speedup 1.0000x reference)
"""Trainium2 Bass kernel for ChronoRotationTransformation.

Computes, per batch row b (B=8192, D=2048):
    u   = (head_r + i*head_i) * (rel_r + i*rel_i)          # complex product
    ab  = sum_d u_r*tail_r - u_i*tail_i                    # == sum rot_r*t_r + rot_i*t_i
    aa  = sum_d u_r^2 + u_i^2                              # == |rot|^2
    bb  = sum_d tail_r^2 + tail_i^2
    out = ab / sqrt(aa*bb)

(The reference's rot = conj(head*rel); rot_r = u_r, rot_i = -u_i, so
ab = rot_r*t_r + rot_i*t_i = u_r*t_r - u_i*t_i and |rot|^2 = |u|^2.)

Sharding: pure data-parallel across 8 NeuronCores, 1024 rows each.
Per core: 8 row-tiles of [128, 2048]. DVE does the 4 cross products,
the two add/subs forming u, and two fused multiply+reduce (ab); ACT
does 4 square+accumulate reductions (aa, bb). Memory-bound target:
~48 MiB HBM reads per core.
"""

import numpy as np

B, D = 8192, 2048
NCORES = 8
BC = B // NCORES            # rows per core
P = 128                     # SBUF partitions
NT = BC // P                # row-tiles per core

IN_NAMES = [
    "head_real", "head_imag",
    "rel_real", "rel_imag",
    "tail_real", "tail_imag",
]

_CACHE = {}


def _emit_v6(tc, ins, out_ap, mybir, repeats=1, cfg="v6"):
    """v6: single-shot tail optimization.

    - Preload the 'sqrt_and_others' ACT table at kernel start (it also
      contains square at 1 ULP) so no LoadActFuncSet lands in the tail.
    - Last row-tile is D-chunked into 4x512 so the DVE chain left after
      the final DMA byte is ~4x shorter; chunk DMAs ordered rel, head,
      tail so product ops unblock earliest.
    - Chunk partial stats share [P, 2*NCH] tiles per stat pair; 3
      tensor_reduce ops fold them straight into column NT-1.
    - Final combine uses 1 Newton step for sqrt (error ~3e-5, far under
      the 2e-2 gate).
    """
    nc = tc.nc
    f32 = mybir.dt.float32
    Alu = mybir.AluOpType
    Act = mybir.ActivationFunctionType

    NCH = 4
    CW = D // NCH
    LAST = NT - 1

    dv = {n: ins[n].rearrange("(t p) d -> t p d", p=P) for n in IN_NAMES}
    out_d = out_ap.rearrange("(t p) -> p t", p=P)

    # Chunk DMA arrival order: products need rel+head first, tail last.
    CH_ORDER = ["rel_real", "rel_imag", "head_real", "head_imag",
                "tail_real", "tail_imag"]

    with (
        tc.tile_pool(name="inp", bufs=2) as inp,
        tc.tile_pool(name="prod", bufs=1) as prod,
        tc.tile_pool(name="upool", bufs=2) as upool,
        tc.tile_pool(name="scr", bufs=1) as scr,
        tc.tile_pool(name="stats", bufs=1) as stats,
    ):
        ab1_s = stats.tile([P, NT], f32, tag="ab1_s")
        ab2_s = stats.tile([P, NT], f32, tag="ab2_s")
        aa1_s = stats.tile([P, NT], f32, tag="aa1_s")
        aa2_s = stats.tile([P, NT], f32, tag="aa2_s")
        bb1_s = stats.tile([P, NT], f32, tag="bb1_s")
        bb2_s = stats.tile([P, NT], f32, tag="bb2_s")
        # Chunk partials for the last tile: halves of each stat pair share
        # one [P, 2*NCH] tile so one reduce folds both.
        abc = stats.tile([P, 2 * NCH], f32, tag="abc")
        aac = stats.tile([P, 2 * NCH], f32, tag="aac")
        bbc = stats.tile([P, 2 * NCH], f32, tag="bbc")

        # ACT table preload: Sqrt pulls in 'sqrt_and_others', whose table
        # also serves Square, so the tail's sqrt needs no table load.
        pre = stats.tile([P, 1], f32, tag="pre")
        nc.vector.memset(pre[:], 1.0)
        nc.scalar.activation(out=pre[:], in_=pre[:], func=Act.Sqrt)

        for _rep in range(repeats):
          for t in range(NT - 1):
            tiles = {}
            for n in IN_NAMES:
                nb = 3 if n.startswith("tail") else 2
                tl = inp.tile([P, D], f32, tag=n, bufs=nb)
                nc.sync.dma_start(out=tl[:], in_=dv[n][t])
                tiles[n] = tl
            hr, hi = tiles["head_real"], tiles["head_imag"]
            rr, ri = tiles["rel_real"], tiles["rel_imag"]
            tr, ti = tiles["tail_real"], tiles["tail_imag"]

            m3 = prod.tile([P, D], f32, tag="m3")
            nc.vector.tensor_mul(m3[:], hi[:], rr[:])
            m4 = prod.tile([P, D], f32, tag="m4")
            nc.vector.tensor_mul(m4[:], hr[:], ri[:])
            m1 = prod.tile([P, D], f32, tag="m1")
            nc.vector.tensor_mul(m1[:], hr[:], rr[:])
            m2 = prod.tile([P, D], f32, tag="m2")
            nc.vector.tensor_mul(m2[:], hi[:], ri[:])
            ur = upool.tile([P, D], f32, tag="ur")
            nc.vector.tensor_sub(ur[:], m1[:], m2[:])
            ui = upool.tile([P, D], f32, tag="ui")
            nc.vector.tensor_add(ui[:], m3[:], m4[:])

            so1 = prod.tile([P, D], f32, tag="m1")
            nc.vector.scalar_tensor_tensor(
                out=so1[:], in0=ur[:], scalar=1.0, in1=tr[:],
                op0=Alu.mult, op1=Alu.mult, accum_out=ab1_s[:, t:t + 1],
            )
            so2 = prod.tile([P, D], f32, tag="m2")
            nc.vector.scalar_tensor_tensor(
                out=so2[:], in0=ui[:], scalar=-1.0, in1=ti[:],
                op0=Alu.mult, op1=Alu.mult, accum_out=ab2_s[:, t:t + 1],
            )

            for src, dst in (
                (tr, bb1_s), (ti, bb2_s), (ur, aa1_s), (ui, aa2_s),
            ):
                ao = scr.tile([P, D], f32, tag="ao")
                nc.scalar.activation(
                    out=ao[:], in_=src[:], func=Act.Square,
                    accum_out=dst[:, t:t + 1],
                )

          # Last tile, D-chunked. Tiles/scratch reuse the same pool tags at
          # full [P, D] size (zero extra SBUF); compute touches [:, :CW].
          for c in range(NCH):
            lo = c * CW
            tiles = {}
            for n in CH_ORDER:
                nb = 3 if n.startswith("tail") else 2
                tl = inp.tile([P, D], f32, tag=n, bufs=nb)
                nc.sync.dma_start(out=tl[:, :CW], in_=dv[n][LAST][:, lo:lo + CW])
                tiles[n] = tl
            hr, hi = tiles["head_real"], tiles["head_imag"]
            rr, ri = tiles["rel_real"], tiles["rel_imag"]
            tr, ti = tiles["tail_real"], tiles["tail_imag"]

            # Emission order = earliest-unblocked first given CH_ORDER.
            m1 = prod.tile([P, D], f32, tag="m1")
            nc.vector.tensor_mul(m1[:, :CW], hr[:, :CW], rr[:, :CW])
            m4 = prod.tile([P, D], f32, tag="m4")
            nc.vector.tensor_mul(m4[:, :CW], hr[:, :CW], ri[:, :CW])
            m3 = prod.tile([P, D], f32, tag="m3")
            nc.vector.tensor_mul(m3[:, :CW], hi[:, :CW], rr[:, :CW])
            m2 = prod.tile([P, D], f32, tag="m2")
            nc.vector.tensor_mul(m2[:, :CW], hi[:, :CW], ri[:, :CW])
            ur = upool.tile([P, D], f32, tag="ur")
            nc.vector.tensor_sub(ur[:, :CW], m1[:, :CW], m2[:, :CW])
            ui = upool.tile([P, D], f32, tag="ui")
            nc.vector.tensor_add(ui[:, :CW], m3[:, :CW], m4[:, :CW])

            so1 = prod.tile([P, D], f32, tag="m1")
            nc.vector.scalar_tensor_tensor(
                out=so1[:, :CW], in0=ur[:, :CW], scalar=1.0, in1=tr[:, :CW],
                op0=Alu.mult, op1=Alu.mult, accum_out=abc[:, c:c + 1],
            )
            so2 = prod.tile([P, D], f32, tag="m2")
            nc.vector.scalar_tensor_tensor(
                out=so2[:, :CW], in0=ui[:, :CW], scalar=-1.0, in1=ti[:, :CW],
                op0=Alu.mult, op1=Alu.mult, accum_out=abc[:, NCH + c:NCH + c + 1],
            )

            for src, dst, col in (
                (tr, bbc, c), (ti, bbc, NCH + c),
                (ur, aac, c), (ui, aac, NCH + c),
            ):
                ao = scr.tile([P, D], f32, tag="ao")
                nc.scalar.activation(
                    out=ao[:, :CW], in_=src[:, :CW], func=Act.Square,
                    accum_out=dst[:, col:col + 1],
                )

          # Final combine. Columns 0..NT-2 come from the full-tile stats;
          # column NT-1 folds straight out of the chunk partials.
          fin = {}
          def ftile(name):
              tl = stats.tile([P, NT], f32, tag=name)
              fin[name] = tl
              return tl

          ab = ftile("ab")
          nc.vector.tensor_add(ab[:, :LAST], ab1_s[:, :LAST], ab2_s[:, :LAST])
          nc.vector.tensor_reduce(
              out=ab[:, LAST:], in_=abc[:], axis=mybir.AxisListType.X,
              op=Alu.add)
          aa = ftile("aa")
          nc.vector.tensor_add(aa[:, :LAST], aa1_s[:, :LAST], aa2_s[:, :LAST])
          nc.vector.tensor_reduce(
              out=aa[:, LAST:], in_=aac[:], axis=mybir.AxisListType.X,
              op=Alu.add)
          bb = ftile("bb")
          nc.vector.tensor_add(bb[:, :LAST], bb1_s[:, :LAST], bb2_s[:, :LAST])
          nc.vector.tensor_reduce(
              out=bb[:, LAST:], in_=bbc[:], axis=mybir.AxisListType.X,
              op=Alu.add)

          pp = ftile("pp"); nc.vector.tensor_mul(pp[:], aa[:], bb[:])
          r = ftile("r0"); nc.scalar.activation(out=r[:], in_=pp[:], func=Act.Sqrt)
          # One Newton step: r <- 0.5*(r + pp/r). ACT sqrt error ~8e-3 rel
          # drops to ~3e-5 — well inside the tolerance.
          q = ftile("q"); nc.vector.reciprocal(q[:], r[:])
          pq = ftile("pq"); nc.vector.tensor_mul(pq[:], pp[:], q[:])
          s = ftile("s"); nc.vector.tensor_add(s[:], r[:], pq[:])
          r1 = ftile("r1"); nc.vector.tensor_scalar_mul(r1[:], s[:], 0.5)
          inv = ftile("inv"); nc.vector.reciprocal(inv[:], r1[:])
          score = ftile("score"); nc.vector.tensor_mul(score[:], ab[:], inv[:])
          nc.sync.dma_start(out=out_d, in_=score[:])


def _emit_v7(tc, ins, out_ap, mybir, repeats=1, cfg="v7"):
    """v7: engine rebalance from HW microbenchmarks (DVE TT 1143ns,
    GPSIMD TT 1636ns, ACT sq+acc 969ns per [128,2048] f32 op; DMA streams
    48 MiB in ~41us — nothing near the old assumed rooflines).

    Packed double-wide tiles: A=(hr|hi), R=(rr|ri), T=(tr|ti), each
    [P, 2D]. Per tile:
      DVE:    m12 = A*R (one [P,2D] mul -> m1|m2)
              u[:, :D]  = m12[:, :D] - m12[:, D:]      (ur)
              STT (u*1)*T accum -> ab column            (one [P,2D] op,
              works because u[:, D:] holds MINUS ui, so the 2D-wide
              accum gives sum(ur*tr) - sum(ui*ti) directly)
      GPSIMD: m34[:, :D] = A[:, D:]*R[:, :D]            (m3 = hi*rr)
              m34[:, D:] = A[:, :D]*R[:, D:]            (m4 = hr*ri)
              u[:, D:]   = (m34[:, :D] * -1) - m34[:, D:]  (-ui)
      ACT:    sq(T) [P,2D] accum -> bb; sq(u) [P,2D] accum -> aa
    Last tile split into NCH chunks (halves packed at width CW) to cut
    the post-DMA tail. Sqrt table preloaded. Single-column stats.
    """
    nc = tc.nc
    f32 = mybir.dt.float32
    Alu = mybir.AluOpType
    Act = mybir.ActivationFunctionType

    NCH = 2
    CW = D // NCH
    LAST = NT - 1
    D2 = 2 * D

    dv = {n: ins[n].rearrange("(t p) d -> t p d", p=P) for n in IN_NAMES}
    out_d = out_ap.rearrange("(t p) -> p t", p=P)

    with (
        tc.tile_pool(name="inp", bufs=2) as inp,
        tc.tile_pool(name="prod", bufs=1) as prod,
        tc.tile_pool(name="gpool", bufs=1) as gpool,
        tc.tile_pool(name="upool", bufs=2) as upool,
        tc.tile_pool(name="scr", bufs=1) as scr,
        tc.tile_pool(name="stats", bufs=1) as stats,
    ):
        ab_s = stats.tile([P, NT], f32, tag="ab_s")
        aa_s = stats.tile([P, NT], f32, tag="aa_s")
        bb_s = stats.tile([P, NT], f32, tag="bb_s")
        abc = stats.tile([P, NCH], f32, tag="abc")
        aac = stats.tile([P, NCH], f32, tag="aac")
        bbc = stats.tile([P, NCH], f32, tag="bbc")

        pre = stats.tile([P, 1], f32, tag="pre")
        nc.vector.memset(pre[:], 1.0)
        nc.scalar.activation(out=pre[:], in_=pre[:], func=Act.Sqrt)

        def emit_block(A, R, T, w, ab_col, aa_col, bb_col):
            """One compute block at half-width w over packed tiles."""
            m12 = prod.tile([P, D2], f32, tag="m12")
            nc.vector.tensor_mul(m12[:, :2 * w], A[:, :2 * w], R[:, :2 * w])
            m34 = gpool.tile([P, D2], f32, tag="m34")
            nc.gpsimd.tensor_mul(m34[:, :w], A[:, w:2 * w], R[:, :w])
            nc.gpsimd.tensor_mul(m34[:, w:2 * w], A[:, :w], R[:, w:2 * w])
            # TensorScalarPtr is not a legal Pool-engine opcode, so GPSIMD
            # takes the plain ur sub (TensorTensor) and DVE forms -ui.
            u = upool.tile([P, D2], f32, tag="u")
            nc.gpsimd.tensor_sub(u[:, :w], m12[:, :w], m12[:, w:2 * w])
            nc.vector.scalar_tensor_tensor(
                out=u[:, w:2 * w], in0=m34[:, :w], scalar=-1.0,
                in1=m34[:, w:2 * w], op0=Alu.mult, op1=Alu.subtract,
            )
            so = prod.tile([P, D2], f32, tag="so")
            nc.vector.scalar_tensor_tensor(
                out=so[:, :2 * w], in0=u[:, :2 * w], scalar=1.0,
                in1=T[:, :2 * w], op0=Alu.mult, op1=Alu.mult,
                accum_out=ab_col,
            )
            ao = scr.tile([P, D2], f32, tag="ao")
            nc.scalar.activation(
                out=ao[:, :2 * w], in_=T[:, :2 * w], func=Act.Square,
                accum_out=bb_col,
            )
            ao2 = scr.tile([P, D2], f32, tag="ao")
            nc.scalar.activation(
                out=ao2[:, :2 * w], in_=u[:, :2 * w], func=Act.Square,
                accum_out=aa_col,
            )

        def load_packed(t, lo, w):
            """DMA the 6 tensors for rows-tile t, cols [lo, lo+w), into
            packed A/R/T tiles (halves at offset w)."""
            A = inp.tile([P, D2], f32, tag="A")
            R = inp.tile([P, D2], f32, tag="R")
            T = inp.tile([P, D2], f32, tag="T")
            for tl, n0, n1 in (
                (A, "head_real", "head_imag"),
                (R, "rel_real", "rel_imag"),
                (T, "tail_real", "tail_imag"),
            ):
                nc.sync.dma_start(out=tl[:, :w], in_=dv[n0][t][:, lo:lo + w])
                nc.sync.dma_start(out=tl[:, w:2 * w], in_=dv[n1][t][:, lo:lo + w])
            return A, R, T

        for _rep in range(repeats):
          for t in range(NT - 1):
            A, R, T = load_packed(t, 0, D)
            emit_block(A, R, T, D,
                       ab_s[:, t:t + 1], aa_s[:, t:t + 1], bb_s[:, t:t + 1])

          for c in range(NCH):
            A, R, T = load_packed(LAST, c * CW, CW)
            emit_block(A, R, T, CW,
                       abc[:, c:c + 1], aac[:, c:c + 1], bbc[:, c:c + 1])

          # Fold chunk partials into the last stats column.
          nc.vector.tensor_reduce(
              out=ab_s[:, LAST:], in_=abc[:], axis=mybir.AxisListType.X,
              op=Alu.add)
          nc.vector.tensor_reduce(
              out=aa_s[:, LAST:], in_=aac[:], axis=mybir.AxisListType.X,
              op=Alu.add)
          nc.vector.tensor_reduce(
              out=bb_s[:, LAST:], in_=bbc[:], axis=mybir.AxisListType.X,
              op=Alu.add)

          fin = {}
          def ftile(name):
              tl = stats.tile([P, NT], f32, tag=name)
              fin[name] = tl
              return tl

          pp = ftile("pp"); nc.vector.tensor_mul(pp[:], aa_s[:], bb_s[:])
          r = ftile("r0"); nc.scalar.activation(out=r[:], in_=pp[:], func=Act.Sqrt)
          q = ftile("q"); nc.vector.reciprocal(q[:], r[:])
          pq = ftile("pq"); nc.vector.tensor_mul(pq[:], pp[:], q[:])
          s = ftile("s"); nc.vector.tensor_add(s[:], r[:], pq[:])
          r1 = ftile("r1"); nc.vector.tensor_scalar_mul(r1[:], s[:], 0.5)
          inv = ftile("inv"); nc.vector.reciprocal(inv[:], r1[:])
          score = ftile("score"); nc.vector.tensor_mul(score[:], ab_s[:], inv[:])
          nc.sync.dma_start(out=out_d, in_=score[:])


def _emit(tc, ins, out_ap, mybir, repeats=1, cfg="v4"):
    import concourse.bass as bass  # noqa: F401

    nc = tc.nc
    f32 = mybir.dt.float32
    Alu = mybir.AluOpType
    Act = mybir.ActivationFunctionType

    # DRAM views: [NT, P, D] row-tiles; out as [P, NT] (row = t*128 + p).
    dv = {n: ins[n].rearrange("(t p) d -> t p d", p=P) for n in IN_NAMES}
    out_d = out_ap.rearrange("(t p) -> p t", p=P)

    with (
        tc.tile_pool(name="inp", bufs=2) as inp,
        tc.tile_pool(name="prod", bufs=1) as prod,
        tc.tile_pool(name="upool", bufs=2) as upool,
        tc.tile_pool(name="scr", bufs=1) as scr,
        tc.tile_pool(name="stats", bufs=1) as stats,
    ):
        ab1_s = stats.tile([P, NT], f32, tag="ab1_s")
        ab2_s = stats.tile([P, NT], f32, tag="ab2_s")
        aa1_s = stats.tile([P, NT], f32, tag="aa1_s")
        aa2_s = stats.tile([P, NT], f32, tag="aa2_s")
        bb1_s = stats.tile([P, NT], f32, tag="bb1_s")
        bb2_s = stats.tile([P, NT], f32, tag="bb2_s")

        for _rep in range(repeats):
          for t in range(NT):
            tiles = {}
            for n in IN_NAMES:
                # tail tiles are the last-released each tile (read by the
                # STT dots at the end) — give them one extra buffer so
                # their next DMA isn't gated on the ring.
                nb = 3 if (cfg == "v5" or n.startswith("tail")) else 2
                tl = inp.tile([P, D], f32, tag=n, bufs=nb)
                nc.sync.dma_start(out=tl[:], in_=dv[n][t])
                tiles[n] = tl
            hr, hi = tiles["head_real"], tiles["head_imag"]
            rr, ri = tiles["rel_real"], tiles["rel_imag"]
            tr, ti = tiles["tail_real"], tiles["tail_imag"]

            # All products on DVE: GPSIMD's fp32 tensor_tensor measured
            # ~4.5x slower than DVE here and coupling it into the tile
            # pipeline made the kernel slower, not faster.
            m3 = prod.tile([P, D], f32, tag="m3")
            nc.vector.tensor_mul(m3[:], hi[:], rr[:])
            m4 = prod.tile([P, D], f32, tag="m4")
            nc.vector.tensor_mul(m4[:], hr[:], ri[:])
            m1 = prod.tile([P, D], f32, tag="m1")
            nc.vector.tensor_mul(m1[:], hr[:], rr[:])
            m2 = prod.tile([P, D], f32, tag="m2")
            nc.vector.tensor_mul(m2[:], hi[:], ri[:])
            ub = 1 if cfg == "v5" else 2
            ur = upool.tile([P, D], f32, tag="ur", bufs=ub)
            nc.vector.tensor_sub(ur[:], m1[:], m2[:])
            ui = upool.tile([P, D], f32, tag="ui", bufs=ub)
            nc.vector.tensor_add(ui[:], m3[:], m4[:])

            # ab = sum(ur*tr) - sum(ui*ti): fused multiply+reduce via
            # scalar_tensor_tensor (out = (in0 op0 scalar) op1 in1,
            # accum_out = sum(out)). tensor_tensor_reduce (native TTR
            # opcode) crashes this terminal's NRT — do not use it.
            # Scratch outs alias the dead m1/m2 slots (same pool tag) —
            # WAR/WAW stay on-engine, zero extra SBUF.
            so1 = prod.tile([P, D], f32, tag="m1")
            nc.vector.scalar_tensor_tensor(
                out=so1[:], in0=ur[:], scalar=1.0, in1=tr[:],
                op0=Alu.mult, op1=Alu.mult, accum_out=ab1_s[:, t:t + 1],
            )
            so2 = prod.tile([P, D], f32, tag="m2")
            nc.vector.scalar_tensor_tensor(
                out=so2[:], in0=ui[:], scalar=-1.0, in1=ti[:],
                op0=Alu.mult, op1=Alu.mult, accum_out=ab2_s[:, t:t + 1],
            )

            # aa, bb: square+accumulate on ACT. bb first — tr/ti are
            # already resident before DVE finishes the products, so ACT
            # starts early and tr/ti stay hot for the STT dots.
            for src, dst in (
                (tr, bb1_s), (ti, bb2_s), (ur, aa1_s), (ui, aa2_s),
            ):
                ao = scr.tile([P, D], f32, tag="ao")
                nc.scalar.activation(
                    out=ao[:], in_=src[:], func=Act.Square,
                    accum_out=dst[:, t:t + 1],
                )

        # Final combine on [P, NT] (tiny).
        fin = {}
        def ftile(name):
            tl = stats.tile([P, NT], f32, tag=name)
            fin[name] = tl
            return tl

        ab = ftile("ab"); nc.vector.tensor_add(ab[:], ab1_s[:], ab2_s[:])
        aa = ftile("aa"); nc.vector.tensor_add(aa[:], aa1_s[:], aa2_s[:])
        bb = ftile("bb"); nc.vector.tensor_add(bb[:], bb1_s[:], bb2_s[:])
        pp = ftile("pp"); nc.vector.tensor_mul(pp[:], aa[:], bb[:])
        # sqrt on ACT is low precision (up to ~65536 ULP budget); refine
        # with two Newton iterations  r <- 0.5*(r + p/r)  using the
        # bit-exact DVE reciprocal.
        r = ftile("r0"); nc.scalar.activation(out=r[:], in_=pp[:], func=Act.Sqrt)
        for it in range(2):
            q = ftile(f"q{it}"); nc.vector.reciprocal(q[:], r[:])
            pq = ftile(f"pq{it}"); nc.vector.tensor_mul(pq[:], pp[:], q[:])
            s = ftile(f"s{it}"); nc.vector.tensor_add(s[:], r[:], pq[:])
            r = ftile(f"r{it + 1}"); nc.vector.tensor_scalar_mul(r[:], s[:], 0.5)
        inv = ftile("inv"); nc.vector.reciprocal(inv[:], r[:])
        score = ftile("score"); nc.vector.tensor_mul(score[:], ab[:], inv[:])
        nc.sync.dma_start(out=out_d, in_=score[:])


def _build(repeats=1, cfg="v7"):
    key = ("nc", repeats, cfg)
    if key in _CACHE:
        return _CACHE[key]
    import concourse.tile as tile
    from concourse import bacc, mybir

    # NOTE: num_devices is deliberately NOT set — it enables collective
    # global-comm setup that breaks plain SPMD input binding under the
    # axon/PJRT path (outputs come back as garbage).
    nc = bacc.Bacc(
        "TRN2",
        target_bir_lowering=False,
        debug=False,
    )
    ins = {
        n: nc.dram_tensor(n, [BC, D], mybir.dt.float32, kind="ExternalInput").ap()
        for n in IN_NAMES
    }
    out = nc.dram_tensor("out", [BC], mybir.dt.float32, kind="ExternalOutput").ap()
    if cfg.startswith("v7"):
        emit = _emit_v7
    elif cfg.startswith("v6"):
        emit = _emit_v6
    else:
        emit = _emit
    with tile.TileContext(nc) as tc:
        emit(tc, ins, out, mybir, repeats=repeats, cfg=cfg)
    nc.compile()
    _CACHE[key] = nc
    return nc


def run(inputs, trace=False, **kwargs):
    """Run on 8 cores; returns (full_output, BassKernelResults)."""
    from concourse.bass_utils import run_bass_kernel_spmd

    nc = _build()
    core_ids = list(range(NCORES))
    in_maps = []
    for c in range(NCORES):
        sl = slice(c * BC, (c + 1) * BC)
        in_maps.append(
            {n: np.ascontiguousarray(inputs[n][sl], dtype=np.float32)
             for n in IN_NAMES}
        )
    # The terminal occasionally reports the accelerator unrecoverable
    # (e.g. poisoned by an earlier crashed run); a fresh attempt after a
    # short wait triggers recovery.
    last_exc = None
    for attempt in range(4):
        try:
            res = run_bass_kernel_spmd(nc, in_maps, core_ids, trace=trace, **kwargs)
            break
        except Exception as e:  # noqa: BLE001
            last_exc = e
            if attempt == 3:
                raise
            import time as _time
            _time.sleep(15 * (attempt + 1))
    out = np.concatenate([res.results[c]["out"] for c in range(NCORES)])
    return out.astype(np.float32), res


def kernel(**inputs):
    out, _ = run(inputs)
    return out



# revision 11
# speedup vs baseline: 1.3403x; 1.3403x over previous
"""Trainium2 Bass kernel for ChronoRotationTransformation.

Computes, per batch row b (B=8192, D=2048):
    u   = (head_r + i*head_i) * (rel_r + i*rel_i)          # complex product
    ab  = sum_d u_r*tail_r - u_i*tail_i                    # == sum rot_r*t_r + rot_i*t_i
    aa  = sum_d u_r^2 + u_i^2                              # == |rot|^2
    bb  = sum_d tail_r^2 + tail_i^2
    out = ab / sqrt(aa*bb)

(The reference's rot = conj(head*rel); rot_r = u_r, rot_i = -u_i, so
ab = rot_r*t_r + rot_i*t_i = u_r*t_r - u_i*t_i and |rot|^2 = |u|^2.)

Sharding: pure data-parallel across 8 NeuronCores, 1024 rows each.
Per core: 8 row-tiles of [128, 2048]. DVE does the 4 cross products,
the two add/subs forming u, and two fused multiply+reduce (ab); ACT
does 4 square+accumulate reductions (aa, bb). Memory-bound target:
~48 MiB HBM reads per core.
"""

import numpy as np

B, D = 8192, 2048
NCORES = 8
BC = B // NCORES            # rows per core
P = 128                     # SBUF partitions
NT = BC // P                # row-tiles per core

IN_NAMES = [
    "head_real", "head_imag",
    "rel_real", "rel_imag",
    "tail_real", "tail_imag",
]

_CACHE = {}


def _emit_v6(tc, ins, out_ap, mybir, repeats=1, cfg="v6"):
    """v6: single-shot tail optimization.

    - Preload the 'sqrt_and_others' ACT table at kernel start (it also
      contains square at 1 ULP) so no LoadActFuncSet lands in the tail.
    - Last row-tile is D-chunked into 4x512 so the DVE chain left after
      the final DMA byte is ~4x shorter; chunk DMAs ordered rel, head,
      tail so product ops unblock earliest.
    - Chunk partial stats share [P, 2*NCH] tiles per stat pair; 3
      tensor_reduce ops fold them straight into column NT-1.
    - Final combine uses 1 Newton step for sqrt (error ~3e-5, far under
      the 2e-2 gate).
    """
    nc = tc.nc
    f32 = mybir.dt.float32
    Alu = mybir.AluOpType
    Act = mybir.ActivationFunctionType

    NCH = 4
    CW = D // NCH
    LAST = NT - 1

    dv = {n: ins[n].rearrange("(t p) d -> t p d", p=P) for n in IN_NAMES}
    out_d = out_ap.rearrange("(t p) -> p t", p=P)

    # Chunk DMA arrival order: products need rel+head first, tail last.
    CH_ORDER = ["rel_real", "rel_imag", "head_real", "head_imag",
                "tail_real", "tail_imag"]

    with (
        tc.tile_pool(name="inp", bufs=2) as inp,
        tc.tile_pool(name="prod", bufs=1) as prod,
        tc.tile_pool(name="upool", bufs=2) as upool,
        tc.tile_pool(name="scr", bufs=1) as scr,
        tc.tile_pool(name="stats", bufs=1) as stats,
    ):
        ab1_s = stats.tile([P, NT], f32, tag="ab1_s")
        ab2_s = stats.tile([P, NT], f32, tag="ab2_s")
        aa1_s = stats.tile([P, NT], f32, tag="aa1_s")
        aa2_s = stats.tile([P, NT], f32, tag="aa2_s")
        bb1_s = stats.tile([P, NT], f32, tag="bb1_s")
        bb2_s = stats.tile([P, NT], f32, tag="bb2_s")
        # Chunk partials for the last tile: halves of each stat pair share
        # one [P, 2*NCH] tile so one reduce folds both.
        abc = stats.tile([P, 2 * NCH], f32, tag="abc")
        aac = stats.tile([P, 2 * NCH], f32, tag="aac")
        bbc = stats.tile([P, 2 * NCH], f32, tag="bbc")

        # ACT table preload: Sqrt pulls in 'sqrt_and_others', whose table
        # also serves Square, so the tail's sqrt needs no table load.
        pre = stats.tile([P, 1], f32, tag="pre")
        nc.vector.memset(pre[:], 1.0)
        nc.scalar.activation(out=pre[:], in_=pre[:], func=Act.Sqrt)

        for _rep in range(repeats):
          for t in range(NT - 1):
            tiles = {}
            for n in IN_NAMES:
                nb = 3 if n.startswith("tail") else 2
                tl = inp.tile([P, D], f32, tag=n, bufs=nb)
                nc.sync.dma_start(out=tl[:], in_=dv[n][t])
                tiles[n] = tl
            hr, hi = tiles["head_real"], tiles["head_imag"]
            rr, ri = tiles["rel_real"], tiles["rel_imag"]
            tr, ti = tiles["tail_real"], tiles["tail_imag"]

            m3 = prod.tile([P, D], f32, tag="m3")
            nc.vector.tensor_mul(m3[:], hi[:], rr[:])
            m4 = prod.tile([P, D], f32, tag="m4")
            nc.vector.tensor_mul(m4[:], hr[:], ri[:])
            m1 = prod.tile([P, D], f32, tag="m1")
            nc.vector.tensor_mul(m1[:], hr[:], rr[:])
            m2 = prod.tile([P, D], f32, tag="m2")
            nc.vector.tensor_mul(m2[:], hi[:], ri[:])
            ur = upool.tile([P, D], f32, tag="ur")
            nc.vector.tensor_sub(ur[:], m1[:], m2[:])
            ui = upool.tile([P, D], f32, tag="ui")
            nc.vector.tensor_add(ui[:], m3[:], m4[:])

            so1 = prod.tile([P, D], f32, tag="m1")
            nc.vector.scalar_tensor_tensor(
                out=so1[:], in0=ur[:], scalar=1.0, in1=tr[:],
                op0=Alu.mult, op1=Alu.mult, accum_out=ab1_s[:, t:t + 1],
            )
            so2 = prod.tile([P, D], f32, tag="m2")
            nc.vector.scalar_tensor_tensor(
                out=so2[:], in0=ui[:], scalar=-1.0, in1=ti[:],
                op0=Alu.mult, op1=Alu.mult, accum_out=ab2_s[:, t:t + 1],
            )

            for src, dst in (
                (tr, bb1_s), (ti, bb2_s), (ur, aa1_s), (ui, aa2_s),
            ):
                ao = scr.tile([P, D], f32, tag="ao")
                nc.scalar.activation(
                    out=ao[:], in_=src[:], func=Act.Square,
                    accum_out=dst[:, t:t + 1],
                )

          # Last tile, D-chunked. Tiles/scratch reuse the same pool tags at
          # full [P, D] size (zero extra SBUF); compute touches [:, :CW].
          for c in range(NCH):
            lo = c * CW
            tiles = {}
            for n in CH_ORDER:
                nb = 3 if n.startswith("tail") else 2
                tl = inp.tile([P, D], f32, tag=n, bufs=nb)
                nc.sync.dma_start(out=tl[:, :CW], in_=dv[n][LAST][:, lo:lo + CW])
                tiles[n] = tl
            hr, hi = tiles["head_real"], tiles["head_imag"]
            rr, ri = tiles["rel_real"], tiles["rel_imag"]
            tr, ti = tiles["tail_real"], tiles["tail_imag"]

            # Emission order = earliest-unblocked first given CH_ORDER.
            m1 = prod.tile([P, D], f32, tag="m1")
            nc.vector.tensor_mul(m1[:, :CW], hr[:, :CW], rr[:, :CW])
            m4 = prod.tile([P, D], f32, tag="m4")
            nc.vector.tensor_mul(m4[:, :CW], hr[:, :CW], ri[:, :CW])
            m3 = prod.tile([P, D], f32, tag="m3")
            nc.vector.tensor_mul(m3[:, :CW], hi[:, :CW], rr[:, :CW])
            m2 = prod.tile([P, D], f32, tag="m2")
            nc.vector.tensor_mul(m2[:, :CW], hi[:, :CW], ri[:, :CW])
            ur = upool.tile([P, D], f32, tag="ur")
            nc.vector.tensor_sub(ur[:, :CW], m1[:, :CW], m2[:, :CW])
            ui = upool.tile([P, D], f32, tag="ui")
            nc.vector.tensor_add(ui[:, :CW], m3[:, :CW], m4[:, :CW])

            so1 = prod.tile([P, D], f32, tag="m1")
            nc.vector.scalar_tensor_tensor(
                out=so1[:, :CW], in0=ur[:, :CW], scalar=1.0, in1=tr[:, :CW],
                op0=Alu.mult, op1=Alu.mult, accum_out=abc[:, c:c + 1],
            )
            so2 = prod.tile([P, D], f32, tag="m2")
            nc.vector.scalar_tensor_tensor(
                out=so2[:, :CW], in0=ui[:, :CW], scalar=-1.0, in1=ti[:, :CW],
                op0=Alu.mult, op1=Alu.mult, accum_out=abc[:, NCH + c:NCH + c + 1],
            )

            for src, dst, col in (
                (tr, bbc, c), (ti, bbc, NCH + c),
                (ur, aac, c), (ui, aac, NCH + c),
            ):
                ao = scr.tile([P, D], f32, tag="ao")
                nc.scalar.activation(
                    out=ao[:, :CW], in_=src[:, :CW], func=Act.Square,
                    accum_out=dst[:, col:col + 1],
                )

          # Final combine. Columns 0..NT-2 come from the full-tile stats;
          # column NT-1 folds straight out of the chunk partials.
          fin = {}
          def ftile(name):
              tl = stats.tile([P, NT], f32, tag=name)
              fin[name] = tl
              return tl

          ab = ftile("ab")
          nc.vector.tensor_add(ab[:, :LAST], ab1_s[:, :LAST], ab2_s[:, :LAST])
          nc.vector.tensor_reduce(
              out=ab[:, LAST:], in_=abc[:], axis=mybir.AxisListType.X,
              op=Alu.add)
          aa = ftile("aa")
          nc.vector.tensor_add(aa[:, :LAST], aa1_s[:, :LAST], aa2_s[:, :LAST])
          nc.vector.tensor_reduce(
              out=aa[:, LAST:], in_=aac[:], axis=mybir.AxisListType.X,
              op=Alu.add)
          bb = ftile("bb")
          nc.vector.tensor_add(bb[:, :LAST], bb1_s[:, :LAST], bb2_s[:, :LAST])
          nc.vector.tensor_reduce(
              out=bb[:, LAST:], in_=bbc[:], axis=mybir.AxisListType.X,
              op=Alu.add)

          pp = ftile("pp"); nc.vector.tensor_mul(pp[:], aa[:], bb[:])
          r = ftile("r0"); nc.scalar.activation(out=r[:], in_=pp[:], func=Act.Sqrt)
          # One Newton step: r <- 0.5*(r + pp/r). ACT sqrt error ~8e-3 rel
          # drops to ~3e-5 — well inside the tolerance.
          q = ftile("q"); nc.vector.reciprocal(q[:], r[:])
          pq = ftile("pq"); nc.vector.tensor_mul(pq[:], pp[:], q[:])
          s = ftile("s"); nc.vector.tensor_add(s[:], r[:], pq[:])
          r1 = ftile("r1"); nc.vector.tensor_scalar_mul(r1[:], s[:], 0.5)
          inv = ftile("inv"); nc.vector.reciprocal(inv[:], r1[:])
          score = ftile("score"); nc.vector.tensor_mul(score[:], ab[:], inv[:])
          nc.sync.dma_start(out=out_d, in_=score[:])


def _emit_v7(tc, ins, out_ap, mybir, repeats=1, cfg="v7"):
    """v7: engine rebalance from HW microbenchmarks (DVE TT 1143ns,
    GPSIMD TT 1636ns, ACT sq+acc 969ns per [128,2048] f32 op; DMA streams
    48 MiB in ~41us — nothing near the old assumed rooflines).

    Packed double-wide tiles: A=(hr|hi), R=(rr|ri), T=(tr|ti), each
    [P, 2D]. Per tile:
      DVE:    m12 = A*R (one [P,2D] mul -> m1|m2)
              u[:, :D]  = m12[:, :D] - m12[:, D:]      (ur)
              STT (u*1)*T accum -> ab column            (one [P,2D] op,
              works because u[:, D:] holds MINUS ui, so the 2D-wide
              accum gives sum(ur*tr) - sum(ui*ti) directly)
      GPSIMD: m34[:, :D] = A[:, D:]*R[:, :D]            (m3 = hi*rr)
              m34[:, D:] = A[:, :D]*R[:, D:]            (m4 = hr*ri)
              u[:, D:]   = (m34[:, :D] * -1) - m34[:, D:]  (-ui)
      ACT:    sq(T) [P,2D] accum -> bb; sq(u) [P,2D] accum -> aa
    Last tile split into NCH chunks (halves packed at width CW) to cut
    the post-DMA tail. Sqrt table preloaded. Single-column stats.
    """
    nc = tc.nc
    f32 = mybir.dt.float32
    Alu = mybir.AluOpType
    Act = mybir.ActivationFunctionType

    # DVE (not DMA) paces this kernel, so last-tile chunking only adds
    # instruction overhead; cfg "v7c2" re-enables it for A/B tests.
    NCH = 2 if cfg == "v7c2" else 1
    CW = D // NCH
    LAST = NT - 1
    D2 = 2 * D

    dv = {n: ins[n].rearrange("(t p) d -> t p d", p=P) for n in IN_NAMES}
    out_d = out_ap.rearrange("(t p) -> p t", p=P)

    with (
        tc.tile_pool(name="inp", bufs=2) as inp,
        tc.tile_pool(name="prod", bufs=1) as prod,
        tc.tile_pool(name="gpool", bufs=1) as gpool,
        tc.tile_pool(name="upool", bufs=2) as upool,
        tc.tile_pool(name="scr", bufs=1) as scr,
        tc.tile_pool(name="stats", bufs=1) as stats,
    ):
        ab_s = stats.tile([P, NT], f32, tag="ab_s")
        aa_s = stats.tile([P, NT], f32, tag="aa_s")
        bb_s = stats.tile([P, NT], f32, tag="bb_s")
        abc = stats.tile([P, NCH], f32, tag="abc")
        aac = stats.tile([P, NCH], f32, tag="aac")
        bbc = stats.tile([P, NCH], f32, tag="bbc")

        pre = stats.tile([P, 1], f32, tag="pre")
        nc.vector.memset(pre[:], 1.0)
        nc.scalar.activation(out=pre[:], in_=pre[:], func=Act.Sqrt)

        def emit_block(A, R, T, w, ab_col, aa_col, bb_col):
            """One compute block at half-width w over packed tiles."""
            m12 = prod.tile([P, D2], f32, tag="m12")
            nc.vector.tensor_mul(m12[:, :2 * w], A[:, :2 * w], R[:, :2 * w])
            m34 = gpool.tile([P, D2], f32, tag="m34")
            nc.vector.tensor_mul(m34[:, :w], A[:, w:2 * w], R[:, :w])
            nc.vector.tensor_mul(m34[:, w:2 * w], A[:, :w], R[:, w:2 * w])
            u = upool.tile([P, D2], f32, tag="u")
            nc.vector.tensor_sub(u[:, :w], m12[:, :w], m12[:, w:2 * w])
            nc.vector.scalar_tensor_tensor(
                out=u[:, w:2 * w], in0=m34[:, :w], scalar=-1.0,
                in1=m34[:, w:2 * w], op0=Alu.mult, op1=Alu.subtract,
            )
            so = prod.tile([P, D2], f32, tag="so")
            nc.vector.scalar_tensor_tensor(
                out=so[:, :2 * w], in0=u[:, :2 * w], scalar=1.0,
                in1=T[:, :2 * w], op0=Alu.mult, op1=Alu.mult,
                accum_out=ab_col,
            )
            ao = scr.tile([P, D2], f32, tag="ao")
            nc.scalar.activation(
                out=ao[:, :2 * w], in_=T[:, :2 * w], func=Act.Square,
                accum_out=bb_col,
            )
            ao2 = scr.tile([P, D2], f32, tag="ao")
            nc.scalar.activation(
                out=ao2[:, :2 * w], in_=u[:, :2 * w], func=Act.Square,
                accum_out=aa_col,
            )

        def load_packed(t, lo, w):
            """DMA the 6 tensors for rows-tile t, cols [lo, lo+w), into
            packed A/R/T tiles (halves at offset w)."""
            A = inp.tile([P, D2], f32, tag="A")
            R = inp.tile([P, D2], f32, tag="R")
            T = inp.tile([P, D2], f32, tag="T")
            for tl, n0, n1 in (
                (A, "head_real", "head_imag"),
                (R, "rel_real", "rel_imag"),
                (T, "tail_real", "tail_imag"),
            ):
                nc.sync.dma_start(out=tl[:, :w], in_=dv[n0][t][:, lo:lo + w])
                nc.sync.dma_start(out=tl[:, w:2 * w], in_=dv[n1][t][:, lo:lo + w])
            return A, R, T

        for _rep in range(repeats):
          nfull = NT if NCH == 1 else NT - 1
          for t in range(nfull):
            A, R, T = load_packed(t, 0, D)
            emit_block(A, R, T, D,
                       ab_s[:, t:t + 1], aa_s[:, t:t + 1], bb_s[:, t:t + 1])

          if NCH > 1:
            for c in range(NCH):
              A, R, T = load_packed(LAST, c * CW, CW)
              emit_block(A, R, T, CW,
                         abc[:, c:c + 1], aac[:, c:c + 1], bbc[:, c:c + 1])

            # Fold chunk partials into the last stats column.
            nc.vector.tensor_reduce(
                out=ab_s[:, LAST:], in_=abc[:], axis=mybir.AxisListType.X,
                op=Alu.add)
            nc.vector.tensor_reduce(
                out=aa_s[:, LAST:], in_=aac[:], axis=mybir.AxisListType.X,
                op=Alu.add)
            nc.vector.tensor_reduce(
                out=bb_s[:, LAST:], in_=bbc[:], axis=mybir.AxisListType.X,
                op=Alu.add)

          fin = {}
          def ftile(name):
              tl = stats.tile([P, NT], f32, tag=name)
              fin[name] = tl
              return tl

          pp = ftile("pp"); nc.vector.tensor_mul(pp[:], aa_s[:], bb_s[:])
          r = ftile("r0"); nc.scalar.activation(out=r[:], in_=pp[:], func=Act.Sqrt)
          q = ftile("q"); nc.vector.reciprocal(q[:], r[:])
          pq = ftile("pq"); nc.vector.tensor_mul(pq[:], pp[:], q[:])
          s = ftile("s"); nc.vector.tensor_add(s[:], r[:], pq[:])
          r1 = ftile("r1"); nc.vector.tensor_scalar_mul(r1[:], s[:], 0.5)
          inv = ftile("inv"); nc.vector.reciprocal(inv[:], r1[:])
          score = ftile("score"); nc.vector.tensor_mul(score[:], ab_s[:], inv[:])
          nc.sync.dma_start(out=out_d, in_=score[:])


def _emit(tc, ins, out_ap, mybir, repeats=1, cfg="v4"):
    import concourse.bass as bass  # noqa: F401

    nc = tc.nc
    f32 = mybir.dt.float32
    Alu = mybir.AluOpType
    Act = mybir.ActivationFunctionType

    # DRAM views: [NT, P, D] row-tiles; out as [P, NT] (row = t*128 + p).
    dv = {n: ins[n].rearrange("(t p) d -> t p d", p=P) for n in IN_NAMES}
    out_d = out_ap.rearrange("(t p) -> p t", p=P)

    with (
        tc.tile_pool(name="inp", bufs=2) as inp,
        tc.tile_pool(name="prod", bufs=1) as prod,
        tc.tile_pool(name="upool", bufs=2) as upool,
        tc.tile_pool(name="scr", bufs=1) as scr,
        tc.tile_pool(name="stats", bufs=1) as stats,
    ):
        ab1_s = stats.tile([P, NT], f32, tag="ab1_s")
        ab2_s = stats.tile([P, NT], f32, tag="ab2_s")
        aa1_s = stats.tile([P, NT], f32, tag="aa1_s")
        aa2_s = stats.tile([P, NT], f32, tag="aa2_s")
        bb1_s = stats.tile([P, NT], f32, tag="bb1_s")
        bb2_s = stats.tile([P, NT], f32, tag="bb2_s")

        for _rep in range(repeats):
          for t in range(NT):
            tiles = {}
            for n in IN_NAMES:
                # tail tiles are the last-released each tile (read by the
                # STT dots at the end) — give them one extra buffer so
                # their next DMA isn't gated on the ring.
                nb = 3 if (cfg == "v5" or n.startswith("tail")) else 2
                tl = inp.tile([P, D], f32, tag=n, bufs=nb)
                nc.sync.dma_start(out=tl[:], in_=dv[n][t])
                tiles[n] = tl
            hr, hi = tiles["head_real"], tiles["head_imag"]
            rr, ri = tiles["rel_real"], tiles["rel_imag"]
            tr, ti = tiles["tail_real"], tiles["tail_imag"]

            # All products on DVE: GPSIMD's fp32 tensor_tensor measured
            # ~4.5x slower than DVE here and coupling it into the tile
            # pipeline made the kernel slower, not faster.
            m3 = prod.tile([P, D], f32, tag="m3")
            nc.vector.tensor_mul(m3[:], hi[:], rr[:])
            m4 = prod.tile([P, D], f32, tag="m4")
            nc.vector.tensor_mul(m4[:], hr[:], ri[:])
            m1 = prod.tile([P, D], f32, tag="m1")
            nc.vector.tensor_mul(m1[:], hr[:], rr[:])
            m2 = prod.tile([P, D], f32, tag="m2")
            nc.vector.tensor_mul(m2[:], hi[:], ri[:])
            ub = 1 if cfg == "v5" else 2
            ur = upool.tile([P, D], f32, tag="ur", bufs=ub)
            nc.vector.tensor_sub(ur[:], m1[:], m2[:])
            ui = upool.tile([P, D], f32, tag="ui", bufs=ub)
            nc.vector.tensor_add(ui[:], m3[:], m4[:])

            # ab = sum(ur*tr) - sum(ui*ti): fused multiply+reduce via
            # scalar_tensor_tensor (out = (in0 op0 scalar) op1 in1,
            # accum_out = sum(out)). tensor_tensor_reduce (native TTR
            # opcode) crashes this terminal's NRT — do not use it.
            # Scratch outs alias the dead m1/m2 slots (same pool tag) —
            # WAR/WAW stay on-engine, zero extra SBUF.
            so1 = prod.tile([P, D], f32, tag="m1")
            nc.vector.scalar_tensor_tensor(
                out=so1[:], in0=ur[:], scalar=1.0, in1=tr[:],
                op0=Alu.mult, op1=Alu.mult, accum_out=ab1_s[:, t:t + 1],
            )
            so2 = prod.tile([P, D], f32, tag="m2")
            nc.vector.scalar_tensor_tensor(
                out=so2[:], in0=ui[:], scalar=-1.0, in1=ti[:],
                op0=Alu.mult, op1=Alu.mult, accum_out=ab2_s[:, t:t + 1],
            )

            # aa, bb: square+accumulate on ACT. bb first — tr/ti are
            # already resident before DVE finishes the products, so ACT
            # starts early and tr/ti stay hot for the STT dots.
            for src, dst in (
                (tr, bb1_s), (ti, bb2_s), (ur, aa1_s), (ui, aa2_s),
            ):
                ao = scr.tile([P, D], f32, tag="ao")
                nc.scalar.activation(
                    out=ao[:], in_=src[:], func=Act.Square,
                    accum_out=dst[:, t:t + 1],
                )

        # Final combine on [P, NT] (tiny).
        fin = {}
        def ftile(name):
            tl = stats.tile([P, NT], f32, tag=name)
            fin[name] = tl
            return tl

        ab = ftile("ab"); nc.vector.tensor_add(ab[:], ab1_s[:], ab2_s[:])
        aa = ftile("aa"); nc.vector.tensor_add(aa[:], aa1_s[:], aa2_s[:])
        bb = ftile("bb"); nc.vector.tensor_add(bb[:], bb1_s[:], bb2_s[:])
        pp = ftile("pp"); nc.vector.tensor_mul(pp[:], aa[:], bb[:])
        # sqrt on ACT is low precision (up to ~65536 ULP budget); refine
        # with two Newton iterations  r <- 0.5*(r + p/r)  using the
        # bit-exact DVE reciprocal.
        r = ftile("r0"); nc.scalar.activation(out=r[:], in_=pp[:], func=Act.Sqrt)
        for it in range(2):
            q = ftile(f"q{it}"); nc.vector.reciprocal(q[:], r[:])
            pq = ftile(f"pq{it}"); nc.vector.tensor_mul(pq[:], pp[:], q[:])
            s = ftile(f"s{it}"); nc.vector.tensor_add(s[:], r[:], pq[:])
            r = ftile(f"r{it + 1}"); nc.vector.tensor_scalar_mul(r[:], s[:], 0.5)
        inv = ftile("inv"); nc.vector.reciprocal(inv[:], r[:])
        score = ftile("score"); nc.vector.tensor_mul(score[:], ab[:], inv[:])
        nc.sync.dma_start(out=out_d, in_=score[:])


def _build(repeats=1, cfg="v7"):
    key = ("nc", repeats, cfg)
    if key in _CACHE:
        return _CACHE[key]
    import concourse.tile as tile
    from concourse import bacc, mybir

    # NOTE: num_devices is deliberately NOT set — it enables collective
    # global-comm setup that breaks plain SPMD input binding under the
    # axon/PJRT path (outputs come back as garbage).
    nc = bacc.Bacc(
        "TRN2",
        target_bir_lowering=False,
        debug=False,
    )
    ins = {
        n: nc.dram_tensor(n, [BC, D], mybir.dt.float32, kind="ExternalInput").ap()
        for n in IN_NAMES
    }
    out = nc.dram_tensor("out", [BC], mybir.dt.float32, kind="ExternalOutput").ap()
    if cfg.startswith("v7"):
        emit = _emit_v7
    elif cfg.startswith("v6"):
        emit = _emit_v6
    else:
        emit = _emit
    with tile.TileContext(nc) as tc:
        emit(tc, ins, out, mybir, repeats=repeats, cfg=cfg)
    nc.compile()
    _CACHE[key] = nc
    return nc


def run(inputs, trace=False, **kwargs):
    """Run on 8 cores; returns (full_output, BassKernelResults)."""
    from concourse.bass_utils import run_bass_kernel_spmd

    nc = _build()
    core_ids = list(range(NCORES))
    in_maps = []
    for c in range(NCORES):
        sl = slice(c * BC, (c + 1) * BC)
        in_maps.append(
            {n: np.ascontiguousarray(inputs[n][sl], dtype=np.float32)
             for n in IN_NAMES}
        )
    # The terminal occasionally reports the accelerator unrecoverable
    # (e.g. poisoned by an earlier crashed run); a fresh attempt after a
    # short wait triggers recovery.
    last_exc = None
    for attempt in range(4):
        try:
            res = run_bass_kernel_spmd(nc, in_maps, core_ids, trace=trace, **kwargs)
            break
        except Exception as e:  # noqa: BLE001
            last_exc = e
            if attempt == 3:
                raise
            import time as _time
            _time.sleep(15 * (attempt + 1))
    out = np.concatenate([res.results[c]["out"] for c in range(NCORES)])
    return out.astype(np.float32), res


def kernel(**inputs):
    out, _ = run(inputs)
    return out



# revision 15
# speedup vs baseline: 1.6048x; 1.1974x over previous
"""Trainium2 Bass kernel for ChronoRotationTransformation.

Computes, per batch row b (B=8192, D=2048):
    u   = (head_r + i*head_i) * (rel_r + i*rel_i)          # complex product
    ab  = sum_d u_r*tail_r - u_i*tail_i                    # == sum rot_r*t_r + rot_i*t_i
    aa  = sum_d u_r^2 + u_i^2                              # == |rot|^2
    bb  = sum_d tail_r^2 + tail_i^2
    out = ab / sqrt(aa*bb)

(The reference's rot = conj(head*rel); rot_r = u_r, rot_i = -u_i, so
ab = rot_r*t_r + rot_i*t_i = u_r*t_r - u_i*t_i and |rot|^2 = |u|^2.)

Sharding: pure data-parallel across 8 NeuronCores, 1024 rows each.
Per core: 8 row-tiles of [128, 2048]. DVE does the 4 cross products,
the two add/subs forming u, and two fused multiply+reduce (ab); ACT
does 4 square+accumulate reductions (aa, bb). Memory-bound target:
~48 MiB HBM reads per core.
"""

import numpy as np

B, D = 8192, 2048
NCORES = 8
BC = B // NCORES            # rows per core
P = 128                     # SBUF partitions
NT = BC // P                # row-tiles per core

IN_NAMES = [
    "head_real", "head_imag",
    "rel_real", "rel_imag",
    "tail_real", "tail_imag",
]

_CACHE = {}


def _emit_v6(tc, ins, out_ap, mybir, repeats=1, cfg="v6"):
    """v6: single-shot tail optimization.

    - Preload the 'sqrt_and_others' ACT table at kernel start (it also
      contains square at 1 ULP) so no LoadActFuncSet lands in the tail.
    - Last row-tile is D-chunked into 4x512 so the DVE chain left after
      the final DMA byte is ~4x shorter; chunk DMAs ordered rel, head,
      tail so product ops unblock earliest.
    - Chunk partial stats share [P, 2*NCH] tiles per stat pair; 3
      tensor_reduce ops fold them straight into column NT-1.
    - Final combine uses 1 Newton step for sqrt (error ~3e-5, far under
      the 2e-2 gate).
    """
    nc = tc.nc
    f32 = mybir.dt.float32
    Alu = mybir.AluOpType
    Act = mybir.ActivationFunctionType

    NCH = 4
    CW = D // NCH
    LAST = NT - 1

    dv = {n: ins[n].rearrange("(t p) d -> t p d", p=P) for n in IN_NAMES}
    out_d = out_ap.rearrange("(t p) -> p t", p=P)

    # Chunk DMA arrival order: products need rel+head first, tail last.
    CH_ORDER = ["rel_real", "rel_imag", "head_real", "head_imag",
                "tail_real", "tail_imag"]

    with (
        tc.tile_pool(name="inp", bufs=2) as inp,
        tc.tile_pool(name="prod", bufs=1) as prod,
        tc.tile_pool(name="upool", bufs=2) as upool,
        tc.tile_pool(name="scr", bufs=1) as scr,
        tc.tile_pool(name="stats", bufs=1) as stats,
    ):
        ab1_s = stats.tile([P, NT], f32, tag="ab1_s")
        ab2_s = stats.tile([P, NT], f32, tag="ab2_s")
        aa1_s = stats.tile([P, NT], f32, tag="aa1_s")
        aa2_s = stats.tile([P, NT], f32, tag="aa2_s")
        bb1_s = stats.tile([P, NT], f32, tag="bb1_s")
        bb2_s = stats.tile([P, NT], f32, tag="bb2_s")
        # Chunk partials for the last tile: halves of each stat pair share
        # one [P, 2*NCH] tile so one reduce folds both.
        abc = stats.tile([P, 2 * NCH], f32, tag="abc")
        aac = stats.tile([P, 2 * NCH], f32, tag="aac")
        bbc = stats.tile([P, 2 * NCH], f32, tag="bbc")

        # ACT table preload: Sqrt pulls in 'sqrt_and_others', whose table
        # also serves Square, so the tail's sqrt needs no table load.
        pre = stats.tile([P, 1], f32, tag="pre")
        nc.vector.memset(pre[:], 1.0)
        nc.scalar.activation(out=pre[:], in_=pre[:], func=Act.Sqrt)

        for _rep in range(repeats):
          for t in range(NT - 1):
            tiles = {}
            for n in IN_NAMES:
                nb = 3 if n.startswith("tail") else 2
                tl = inp.tile([P, D], f32, tag=n, bufs=nb)
                nc.sync.dma_start(out=tl[:], in_=dv[n][t])
                tiles[n] = tl
            hr, hi = tiles["head_real"], tiles["head_imag"]
            rr, ri = tiles["rel_real"], tiles["rel_imag"]
            tr, ti = tiles["tail_real"], tiles["tail_imag"]

            m3 = prod.tile([P, D], f32, tag="m3")
            nc.vector.tensor_mul(m3[:], hi[:], rr[:])
            m4 = prod.tile([P, D], f32, tag="m4")
            nc.vector.tensor_mul(m4[:], hr[:], ri[:])
            m1 = prod.tile([P, D], f32, tag="m1")
            nc.vector.tensor_mul(m1[:], hr[:], rr[:])
            m2 = prod.tile([P, D], f32, tag="m2")
            nc.vector.tensor_mul(m2[:], hi[:], ri[:])
            ur = upool.tile([P, D], f32, tag="ur")
            nc.vector.tensor_sub(ur[:], m1[:], m2[:])
            ui = upool.tile([P, D], f32, tag="ui")
            nc.vector.tensor_add(ui[:], m3[:], m4[:])

            so1 = prod.tile([P, D], f32, tag="m1")
            nc.vector.scalar_tensor_tensor(
                out=so1[:], in0=ur[:], scalar=1.0, in1=tr[:],
                op0=Alu.mult, op1=Alu.mult, accum_out=ab1_s[:, t:t + 1],
            )
            so2 = prod.tile([P, D], f32, tag="m2")
            nc.vector.scalar_tensor_tensor(
                out=so2[:], in0=ui[:], scalar=-1.0, in1=ti[:],
                op0=Alu.mult, op1=Alu.mult, accum_out=ab2_s[:, t:t + 1],
            )

            for src, dst in (
                (tr, bb1_s), (ti, bb2_s), (ur, aa1_s), (ui, aa2_s),
            ):
                ao = scr.tile([P, D], f32, tag="ao")
                nc.scalar.activation(
                    out=ao[:], in_=src[:], func=Act.Square,
                    accum_out=dst[:, t:t + 1],
                )

          # Last tile, D-chunked. Tiles/scratch reuse the same pool tags at
          # full [P, D] size (zero extra SBUF); compute touches [:, :CW].
          for c in range(NCH):
            lo = c * CW
            tiles = {}
            for n in CH_ORDER:
                nb = 3 if n.startswith("tail") else 2
                tl = inp.tile([P, D], f32, tag=n, bufs=nb)
                nc.sync.dma_start(out=tl[:, :CW], in_=dv[n][LAST][:, lo:lo + CW])
                tiles[n] = tl
            hr, hi = tiles["head_real"], tiles["head_imag"]
            rr, ri = tiles["rel_real"], tiles["rel_imag"]
            tr, ti = tiles["tail_real"], tiles["tail_imag"]

            # Emission order = earliest-unblocked first given CH_ORDER.
            m1 = prod.tile([P, D], f32, tag="m1")
            nc.vector.tensor_mul(m1[:, :CW], hr[:, :CW], rr[:, :CW])
            m4 = prod.tile([P, D], f32, tag="m4")
            nc.vector.tensor_mul(m4[:, :CW], hr[:, :CW], ri[:, :CW])
            m3 = prod.tile([P, D], f32, tag="m3")
            nc.vector.tensor_mul(m3[:, :CW], hi[:, :CW], rr[:, :CW])
            m2 = prod.tile([P, D], f32, tag="m2")
            nc.vector.tensor_mul(m2[:, :CW], hi[:, :CW], ri[:, :CW])
            ur = upool.tile([P, D], f32, tag="ur")
            nc.vector.tensor_sub(ur[:, :CW], m1[:, :CW], m2[:, :CW])
            ui = upool.tile([P, D], f32, tag="ui")
            nc.vector.tensor_add(ui[:, :CW], m3[:, :CW], m4[:, :CW])

            so1 = prod.tile([P, D], f32, tag="m1")
            nc.vector.scalar_tensor_tensor(
                out=so1[:, :CW], in0=ur[:, :CW], scalar=1.0, in1=tr[:, :CW],
                op0=Alu.mult, op1=Alu.mult, accum_out=abc[:, c:c + 1],
            )
            so2 = prod.tile([P, D], f32, tag="m2")
            nc.vector.scalar_tensor_tensor(
                out=so2[:, :CW], in0=ui[:, :CW], scalar=-1.0, in1=ti[:, :CW],
                op0=Alu.mult, op1=Alu.mult, accum_out=abc[:, NCH + c:NCH + c + 1],
            )

            for src, dst, col in (
                (tr, bbc, c), (ti, bbc, NCH + c),
                (ur, aac, c), (ui, aac, NCH + c),
            ):
                ao = scr.tile([P, D], f32, tag="ao")
                nc.scalar.activation(
                    out=ao[:, :CW], in_=src[:, :CW], func=Act.Square,
                    accum_out=dst[:, col:col + 1],
                )

          # Final combine. Columns 0..NT-2 come from the full-tile stats;
          # column NT-1 folds straight out of the chunk partials.
          fin = {}
          def ftile(name):
              tl = stats.tile([P, NT], f32, tag=name)
              fin[name] = tl
              return tl

          ab = ftile("ab")
          nc.vector.tensor_add(ab[:, :LAST], ab1_s[:, :LAST], ab2_s[:, :LAST])
          nc.vector.tensor_reduce(
              out=ab[:, LAST:], in_=abc[:], axis=mybir.AxisListType.X,
              op=Alu.add)
          aa = ftile("aa")
          nc.vector.tensor_add(aa[:, :LAST], aa1_s[:, :LAST], aa2_s[:, :LAST])
          nc.vector.tensor_reduce(
              out=aa[:, LAST:], in_=aac[:], axis=mybir.AxisListType.X,
              op=Alu.add)
          bb = ftile("bb")
          nc.vector.tensor_add(bb[:, :LAST], bb1_s[:, :LAST], bb2_s[:, :LAST])
          nc.vector.tensor_reduce(
              out=bb[:, LAST:], in_=bbc[:], axis=mybir.AxisListType.X,
              op=Alu.add)

          pp = ftile("pp"); nc.vector.tensor_mul(pp[:], aa[:], bb[:])
          r = ftile("r0"); nc.scalar.activation(out=r[:], in_=pp[:], func=Act.Sqrt)
          # One Newton step: r <- 0.5*(r + pp/r). ACT sqrt error ~8e-3 rel
          # drops to ~3e-5 — well inside the tolerance.
          q = ftile("q"); nc.vector.reciprocal(q[:], r[:])
          pq = ftile("pq"); nc.vector.tensor_mul(pq[:], pp[:], q[:])
          s = ftile("s"); nc.vector.tensor_add(s[:], r[:], pq[:])
          r1 = ftile("r1"); nc.vector.tensor_scalar_mul(r1[:], s[:], 0.5)
          inv = ftile("inv"); nc.vector.reciprocal(inv[:], r1[:])
          score = ftile("score"); nc.vector.tensor_mul(score[:], ab[:], inv[:])
          nc.sync.dma_start(out=out_d, in_=score[:])


def _emit_v7(tc, ins, out_ap, mybir, repeats=1, cfg="v7"):
    """v7: engine rebalance from HW microbenchmarks (DVE TT 1143ns,
    GPSIMD TT 1636ns, ACT sq+acc 969ns per [128,2048] f32 op; DMA streams
    48 MiB in ~41us — nothing near the old assumed rooflines).

    Packed double-wide tiles: A=(hr|hi), R=(rr|ri), T=(tr|ti), each
    [P, 2D]. Per tile:
      DVE:    m12 = A*R (one [P,2D] mul -> m1|m2)
              u[:, :D]  = m12[:, :D] - m12[:, D:]      (ur)
              STT (u*1)*T accum -> ab column            (one [P,2D] op,
              works because u[:, D:] holds MINUS ui, so the 2D-wide
              accum gives sum(ur*tr) - sum(ui*ti) directly)
      GPSIMD: m34[:, :D] = A[:, D:]*R[:, :D]            (m3 = hi*rr)
              m34[:, D:] = A[:, :D]*R[:, D:]            (m4 = hr*ri)
              u[:, D:]   = (m34[:, :D] * -1) - m34[:, D:]  (-ui)
      ACT:    sq(T) [P,2D] accum -> bb; sq(u) [P,2D] accum -> aa
    Last tile split into NCH chunks (halves packed at width CW) to cut
    the post-DMA tail. Sqrt table preloaded. Single-column stats.
    """
    nc = tc.nc
    f32 = mybir.dt.float32
    Alu = mybir.AluOpType
    Act = mybir.ActivationFunctionType

    # DVE (not DMA) paces this kernel, so last-tile chunking only adds
    # instruction overhead; cfg "v7c2" re-enables it for A/B tests.
    NCH = 2 if cfg == "v7c2" else 1
    CW = D // NCH
    LAST = NT - 1
    D2 = 2 * D

    dv = {n: ins[n].rearrange("(t p) d -> t p d", p=P) for n in IN_NAMES}
    out_d = out_ap.rearrange("(t p) -> p t", p=P)

    with (
        tc.tile_pool(name="inp", bufs=2) as inp,
        tc.tile_pool(name="prod", bufs=1) as prod,
        tc.tile_pool(name="gpool", bufs=1) as gpool,
        tc.tile_pool(name="upool", bufs=2) as upool,
        tc.tile_pool(name="scr", bufs=1) as scr,
        tc.tile_pool(name="stats", bufs=1) as stats,
    ):
        ab_s = stats.tile([P, NT], f32, tag="ab_s")
        aa_s = stats.tile([P, NT], f32, tag="aa_s")
        bb_s = stats.tile([P, NT], f32, tag="bb_s")
        abc = stats.tile([P, NCH], f32, tag="abc")
        aac = stats.tile([P, NCH], f32, tag="aac")
        bbc = stats.tile([P, NCH], f32, tag="bbc")

        pre = stats.tile([P, 1], f32, tag="pre")
        nc.vector.memset(pre[:], 1.0)
        nc.scalar.activation(out=pre[:], in_=pre[:], func=Act.Sqrt)

        def emit_block(A, R, T, w, ab_col, aa_col, bb_col):
            """One compute block at half-width w over packed tiles."""
            m12 = prod.tile([P, D2], f32, tag="m12")
            nc.vector.tensor_mul(m12[:, :2 * w], A[:, :2 * w], R[:, :2 * w])
            m34 = gpool.tile([P, D2], f32, tag="m34")
            nc.vector.tensor_mul(m34[:, :w], A[:, w:2 * w], R[:, :w])
            nc.vector.tensor_mul(m34[:, w:2 * w], A[:, :w], R[:, w:2 * w])
            u = upool.tile([P, D2], f32, tag="u")
            nc.vector.tensor_sub(u[:, :w], m12[:, :w], m12[:, w:2 * w])
            nc.vector.scalar_tensor_tensor(
                out=u[:, w:2 * w], in0=m34[:, :w], scalar=-1.0,
                in1=m34[:, w:2 * w], op0=Alu.mult, op1=Alu.subtract,
            )
            so = prod.tile([P, D2], f32, tag="so")
            nc.vector.scalar_tensor_tensor(
                out=so[:, :2 * w], in0=u[:, :2 * w], scalar=1.0,
                in1=T[:, :2 * w], op0=Alu.mult, op1=Alu.mult,
                accum_out=ab_col,
            )
            ao = scr.tile([P, D2], f32, tag="ao")
            nc.scalar.activation(
                out=ao[:, :2 * w], in_=T[:, :2 * w], func=Act.Square,
                accum_out=bb_col,
            )
            ao2 = scr.tile([P, D2], f32, tag="ao")
            nc.scalar.activation(
                out=ao2[:, :2 * w], in_=u[:, :2 * w], func=Act.Square,
                accum_out=aa_col,
            )

        def load_packed(t, lo, w):
            """DMA the 6 tensors for rows-tile t, cols [lo, lo+w), into
            packed A/R/T tiles (halves at offset w)."""
            A = inp.tile([P, D2], f32, tag="A")
            R = inp.tile([P, D2], f32, tag="R")
            T = inp.tile([P, D2], f32, tag="T")
            for tl, n0, n1 in (
                (A, "head_real", "head_imag"),
                (R, "rel_real", "rel_imag"),
                (T, "tail_real", "tail_imag"),
            ):
                nc.sync.dma_start(out=tl[:, :w], in_=dv[n0][t][:, lo:lo + w])
                nc.sync.dma_start(out=tl[:, w:2 * w], in_=dv[n1][t][:, lo:lo + w])
            return A, R, T

        # cfg "v7x": combine emitted once after all reps (bench-only A/B to
        # isolate the per-rep combine's cost from the tile pipeline).
        combine_last_only = cfg == "v7x"

        def emit_rep():
          nfull = NT if NCH == 1 else NT - 1
          for t in range(nfull):
            A, R, T = load_packed(t, 0, D)
            emit_block(A, R, T, D,
                       ab_s[:, t:t + 1], aa_s[:, t:t + 1], bb_s[:, t:t + 1])

          if NCH > 1:
            for c in range(NCH):
              A, R, T = load_packed(LAST, c * CW, CW)
              emit_block(A, R, T, CW,
                         abc[:, c:c + 1], aac[:, c:c + 1], bbc[:, c:c + 1])

            # Fold chunk partials into the last stats column.
            nc.vector.tensor_reduce(
                out=ab_s[:, LAST:], in_=abc[:], axis=mybir.AxisListType.X,
                op=Alu.add)
            nc.vector.tensor_reduce(
                out=aa_s[:, LAST:], in_=aac[:], axis=mybir.AxisListType.X,
                op=Alu.add)
            nc.vector.tensor_reduce(
                out=bb_s[:, LAST:], in_=bbc[:], axis=mybir.AxisListType.X,
                op=Alu.add)

          if not combine_last_only:
            _emit_combine_v7(nc, stats, ab_s, aa_s, bb_s, out_d, mybir)

        if cfg == "v7l" and repeats > 1:
            # Hardware loop: constant program size regardless of repeats.
            # The inter-iteration all-engine barrier makes each iteration a
            # fresh single-shot run — the slope approximates what a single
            # invocation costs on HW, free of instruction-fetch scaling.
            with tc.For_i(0, repeats, 1):
                emit_rep()
        else:
            for _rep in range(repeats):
                emit_rep()

        if combine_last_only:
            _emit_combine_v7(nc, stats, ab_s, aa_s, bb_s, out_d, mybir)


def _emit_combine_v7(nc, stats, ab_s, aa_s, bb_s, out_d, mybir):
    f32 = mybir.dt.float32
    Act = mybir.ActivationFunctionType

    fin = {}
    def ftile(name):
        tl = stats.tile([P, NT], f32, tag=name)
        fin[name] = tl
        return tl

    pp = ftile("pp"); nc.vector.tensor_mul(pp[:], aa_s[:], bb_s[:])
    r = ftile("r0"); nc.scalar.activation(out=r[:], in_=pp[:], func=Act.Sqrt)
    q = ftile("q"); nc.vector.reciprocal(q[:], r[:])
    pq = ftile("pq"); nc.vector.tensor_mul(pq[:], pp[:], q[:])
    s = ftile("s"); nc.vector.tensor_add(s[:], r[:], pq[:])
    r1 = ftile("r1"); nc.vector.tensor_scalar_mul(r1[:], s[:], 0.5)
    inv = ftile("inv"); nc.vector.reciprocal(inv[:], r1[:])
    score = ftile("score"); nc.vector.tensor_mul(score[:], ab_s[:], inv[:])
    nc.sync.dma_start(out=out_d, in_=score[:])


def _emit(tc, ins, out_ap, mybir, repeats=1, cfg="v4"):
    import concourse.bass as bass  # noqa: F401

    nc = tc.nc
    f32 = mybir.dt.float32
    Alu = mybir.AluOpType
    Act = mybir.ActivationFunctionType

    # DRAM views: [NT, P, D] row-tiles; out as [P, NT] (row = t*128 + p).
    dv = {n: ins[n].rearrange("(t p) d -> t p d", p=P) for n in IN_NAMES}
    out_d = out_ap.rearrange("(t p) -> p t", p=P)

    with (
        tc.tile_pool(name="inp", bufs=2) as inp,
        tc.tile_pool(name="prod", bufs=1) as prod,
        tc.tile_pool(name="upool", bufs=2) as upool,
        tc.tile_pool(name="scr", bufs=1) as scr,
        tc.tile_pool(name="stats", bufs=1) as stats,
    ):
        ab1_s = stats.tile([P, NT], f32, tag="ab1_s")
        ab2_s = stats.tile([P, NT], f32, tag="ab2_s")
        aa1_s = stats.tile([P, NT], f32, tag="aa1_s")
        aa2_s = stats.tile([P, NT], f32, tag="aa2_s")
        bb1_s = stats.tile([P, NT], f32, tag="bb1_s")
        bb2_s = stats.tile([P, NT], f32, tag="bb2_s")

        for _rep in range(repeats):
          for t in range(NT):
            tiles = {}
            for n in IN_NAMES:
                # tail tiles are the last-released each tile (read by the
                # STT dots at the end) — give them one extra buffer so
                # their next DMA isn't gated on the ring.
                nb = 3 if (cfg == "v5" or n.startswith("tail")) else 2
                tl = inp.tile([P, D], f32, tag=n, bufs=nb)
                nc.sync.dma_start(out=tl[:], in_=dv[n][t])
                tiles[n] = tl
            hr, hi = tiles["head_real"], tiles["head_imag"]
            rr, ri = tiles["rel_real"], tiles["rel_imag"]
            tr, ti = tiles["tail_real"], tiles["tail_imag"]

            # All products on DVE: GPSIMD's fp32 tensor_tensor measured
            # ~4.5x slower than DVE here and coupling it into the tile
            # pipeline made the kernel slower, not faster.
            m3 = prod.tile([P, D], f32, tag="m3")
            nc.vector.tensor_mul(m3[:], hi[:], rr[:])
            m4 = prod.tile([P, D], f32, tag="m4")
            nc.vector.tensor_mul(m4[:], hr[:], ri[:])
            m1 = prod.tile([P, D], f32, tag="m1")
            nc.vector.tensor_mul(m1[:], hr[:], rr[:])
            m2 = prod.tile([P, D], f32, tag="m2")
            nc.vector.tensor_mul(m2[:], hi[:], ri[:])
            ub = 1 if cfg == "v5" else 2
            ur = upool.tile([P, D], f32, tag="ur", bufs=ub)
            nc.vector.tensor_sub(ur[:], m1[:], m2[:])
            ui = upool.tile([P, D], f32, tag="ui", bufs=ub)
            nc.vector.tensor_add(ui[:], m3[:], m4[:])

            # ab = sum(ur*tr) - sum(ui*ti): fused multiply+reduce via
            # scalar_tensor_tensor (out = (in0 op0 scalar) op1 in1,
            # accum_out = sum(out)). tensor_tensor_reduce (native TTR
            # opcode) crashes this terminal's NRT — do not use it.
            # Scratch outs alias the dead m1/m2 slots (same pool tag) —
            # WAR/WAW stay on-engine, zero extra SBUF.
            so1 = prod.tile([P, D], f32, tag="m1")
            nc.vector.scalar_tensor_tensor(
                out=so1[:], in0=ur[:], scalar=1.0, in1=tr[:],
                op0=Alu.mult, op1=Alu.mult, accum_out=ab1_s[:, t:t + 1],
            )
            so2 = prod.tile([P, D], f32, tag="m2")
            nc.vector.scalar_tensor_tensor(
                out=so2[:], in0=ui[:], scalar=-1.0, in1=ti[:],
                op0=Alu.mult, op1=Alu.mult, accum_out=ab2_s[:, t:t + 1],
            )

            # aa, bb: square+accumulate on ACT. bb first — tr/ti are
            # already resident before DVE finishes the products, so ACT
            # starts early and tr/ti stay hot for the STT dots.
            for src, dst in (
                (tr, bb1_s), (ti, bb2_s), (ur, aa1_s), (ui, aa2_s),
            ):
                ao = scr.tile([P, D], f32, tag="ao")
                nc.scalar.activation(
                    out=ao[:], in_=src[:], func=Act.Square,
                    accum_out=dst[:, t:t + 1],
                )

        # Final combine on [P, NT] (tiny).
        fin = {}
        def ftile(name):
            tl = stats.tile([P, NT], f32, tag=name)
            fin[name] = tl
            return tl

        ab = ftile("ab"); nc.vector.tensor_add(ab[:], ab1_s[:], ab2_s[:])
        aa = ftile("aa"); nc.vector.tensor_add(aa[:], aa1_s[:], aa2_s[:])
        bb = ftile("bb"); nc.vector.tensor_add(bb[:], bb1_s[:], bb2_s[:])
        pp = ftile("pp"); nc.vector.tensor_mul(pp[:], aa[:], bb[:])
        # sqrt on ACT is low precision (up to ~65536 ULP budget); refine
        # with two Newton iterations  r <- 0.5*(r + p/r)  using the
        # bit-exact DVE reciprocal.
        r = ftile("r0"); nc.scalar.activation(out=r[:], in_=pp[:], func=Act.Sqrt)
        for it in range(2):
            q = ftile(f"q{it}"); nc.vector.reciprocal(q[:], r[:])
            pq = ftile(f"pq{it}"); nc.vector.tensor_mul(pq[:], pp[:], q[:])
            s = ftile(f"s{it}"); nc.vector.tensor_add(s[:], r[:], pq[:])
            r = ftile(f"r{it + 1}"); nc.vector.tensor_scalar_mul(r[:], s[:], 0.5)
        inv = ftile("inv"); nc.vector.reciprocal(inv[:], r[:])
        score = ftile("score"); nc.vector.tensor_mul(score[:], ab[:], inv[:])
        nc.sync.dma_start(out=out_d, in_=score[:])


def _build(repeats=1, cfg="v7"):
    key = ("nc", repeats, cfg)
    if key in _CACHE:
        return _CACHE[key]
    import concourse.tile as tile
    from concourse import bacc, mybir

    # NOTE: num_devices is deliberately NOT set — it enables collective
    # global-comm setup that breaks plain SPMD input binding under the
    # axon/PJRT path (outputs come back as garbage).
    nc = bacc.Bacc(
        "TRN2",
        target_bir_lowering=False,
        debug=False,
    )
    ins = {
        n: nc.dram_tensor(n, [BC, D], mybir.dt.float32, kind="ExternalInput").ap()
        for n in IN_NAMES
    }
    out = nc.dram_tensor("out", [BC], mybir.dt.float32, kind="ExternalOutput").ap()
    if cfg.startswith("v7"):
        emit = _emit_v7
    elif cfg.startswith("v6"):
        emit = _emit_v6
    else:
        emit = _emit
    with tile.TileContext(nc) as tc:
        emit(tc, ins, out, mybir, repeats=repeats, cfg=cfg)
    nc.compile()
    _CACHE[key] = nc
    return nc


def run(inputs, trace=False, **kwargs):
    """Run on 8 cores; returns (full_output, BassKernelResults)."""
    from concourse.bass_utils import run_bass_kernel_spmd

    nc = _build()
    core_ids = list(range(NCORES))
    in_maps = []
    for c in range(NCORES):
        sl = slice(c * BC, (c + 1) * BC)
        in_maps.append(
            {n: np.ascontiguousarray(inputs[n][sl], dtype=np.float32)
             for n in IN_NAMES}
        )
    # The terminal occasionally reports the accelerator unrecoverable
    # (e.g. poisoned by an earlier crashed run); a fresh attempt after a
    # short wait triggers recovery.
    last_exc = None
    for attempt in range(4):
        try:
            res = run_bass_kernel_spmd(nc, in_maps, core_ids, trace=trace, **kwargs)
            break
        except Exception as e:  # noqa: BLE001
            last_exc = e
            if attempt == 3:
                raise
            import time as _time
            _time.sleep(15 * (attempt + 1))
    out = np.concatenate([res.results[c]["out"] for c in range(NCORES)])
    return out.astype(np.float32), res


def kernel(**inputs):
    out, _ = run(inputs)
    return out

